# revision 11
# baseline (speedup 1.0000x reference)
"""CosFormer causal attention — Trainium2 Bass kernel, 8 NeuronCores.

Sharding: core i = (batch b = i//4, head-group g = i%4 covering heads 2g, 2g+1).
Each core computes the qkv projection for its two heads, chunked causal linear
attention (cos/sin feature channels), and a partial output projection over its
128 context channels. The host unshards by summing the 4 per-core partials of
each batch (the output projection's contraction is sharded over heads) and
adding b_out.

Key layout/perf choices:
- fp16 datapath end to end (PSUM accumulation in fp32): 1 cycle/row on the PE
  at every moving size (fp32r needs moving>=256 and pays 4x below), half the
  HBM traffic of fp32, and ~5e-4 relative rounding - far inside the 2e-2 gate.
- Input DMA split across both HWDGE rings (sync + scalar), coalesced into a
  few large transfers ordered critical-first; no SWDGE (gpsimd) traffic.
- PE warmup chain bridges the initial DMA wait so the tensor engine is past
  its DVFS ramp (0.65/1.2/2.4 GHz p-states) when the real matmuls start.
- Per-head q/k features in [feat, t] layout as one [128, T] tile (rows 0:64 =
  relu(.)*cos_t, 64:128 = relu(.)*sin_t), produced by projecting with
  duplicated weight columns (PE cost scales with the moving dim, not M).
- Attention runs in 256-wide query super-chunks: two 128-wide key stripes are
  scored against the full 256 query band and masked. A [2d, d+2] state carries
  the prefix between super-chunks.
- Normalization: norm row -> PE-transpose to a [t, 1] column -> 1-elem/lane
  reciprocal -> applied per-partition to the per-head out-projection partials.

Fully self-contained: hardcodes B=2, T=1024, E=512, H=8.
"""

import math
from contextlib import ExitStack

import numpy as np

import concourse.bass as bass
import concourse.mybir as mybir
import concourse.tile as tile
from concourse.bass_utils import run_bass_kernel_spmd
from concourse.vector_clock import ScopedClock

B, T, E = 2, 1024, 512
H, D = 8, 64
S = 128            # key stripe size
SC = 256           # query super-chunk size
NSC = T // SC      # 4
F32 = mybir.dt.float32
F16 = mybir.dt.float16
F32R = mybir.dt.float32r
EPS = 1e-6
N_WARMUP = 9


def _install_drain_patch():
    """This walrus build rejects a Drain carrying >1 sem wait. Split the
    Tile-exit drain's waits across single-wait SP nops."""
    if getattr(tile.TileContext, "_drain_patch_installed", False):
        return

    def _patched(self, tick_clock, wait_clock):
        nc = self.nc
        pre = nc.sync.nop(nofuse=True)
        wait_clock.add_sem_waits(pre.ins, ScopedClock({None: tick_clock.global_clock}))
        waits = list(pre.ins.sync_info.on_wait or []) if pre.ins.sync_info else []
        if len(waits) > 1:
            pre.ins.sync_info.on_wait = waits[:1]
            for w in waits[1:]:
                n = nc.sync.nop(nofuse=True)
                if n.ins.sync_info is None:
                    n.ins.sync_info = mybir.SyncInfo(on_wait=[w], on_update=[])
                else:
                    n.ins.sync_info.on_wait = [w]
        nc.sync.drain()
        nc.all_engine_barrier()
        popped = nc._tile_sem_poison_stack.pop()
        assert popped is self._sem_poison

    tile.TileContext._drain_and_barrier = _patched
    tile.TileContext._drain_patch_installed = True


def _split_multi_waits(nc):
    """This walrus build only codegens ONE sync-wait command per instruction.
    Move excess waits onto same-engine NoOps inserted just before."""
    ctr = [0]

    def _mk_nop(engine, wait):
        ctr[0] += 1
        return mybir.InstNoOp(
            name=f"I-waitnop{ctr[0]}",
            engine=engine,
            ins=[],
            outs=[],
            sync_info=mybir.SyncInfo(on_wait=[wait], on_update=[]),
        )

    for f in nc.m.functions:
        for bb in f.blocks:
            new_insts = []
            for inst in bb.instructions:
                si = inst.sync_info
                waits = list(si.on_wait) if si and si.on_wait else []
                if len(waits) > 1:
                    for w in waits[:-1]:
                        new_insts.append(_mk_nop(inst.engine, w))
                    si.on_wait = waits[-1:]
                new_insts.append(inst)
            bb.instructions[:] = new_insts


def build_program() -> bass.Bass:
    _install_drain_patch()
    nc = bass.Bass()

    # wqkf: duplicated weight cols [qf_h0 | qf_h1 | kf_h0 | kf_h1], each 128 wide
    xt = nc.declare_dram_parameter("xt", [E, T], F16, isOutput=False)        # x[b].T
    wqkf = nc.declare_dram_parameter("wqkf", [E, 512], F16, isOutput=False)
    wvt = nc.declare_dram_parameter("wvt", [E, 128], F16, isOutput=False)    # [v0 v1].T
    biasblob = nc.declare_dram_parameter("biasblob", [128, 5], F32, isOutput=False)
    csrep = nc.declare_dram_parameter("csrep", [128, T], F16, isOutput=False)  # [cos;sin]
    w2 = nc.declare_dram_parameter("w2", [128, E], F16, isOutput=False)
    identin = nc.declare_dram_parameter("identin", [128, 128], F16, isOutput=False)
    m0in = nc.declare_dram_parameter("m0in", [S, SC], F16, isOutput=False)    # [tri | ones]
    out = nc.declare_dram_parameter("out", [T, E], F16, isOutput=True)

    with tile.TileContext(nc) as tc, ExitStack() as ctx:
        singles = ctx.enter_context(tc.tile_pool(name="singles", bufs=1))
        kf_pool = ctx.enter_context(tc.tile_pool(name="kf", bufs=4))
        atm_pool = ctx.enter_context(tc.tile_pool(name="atm", bufs=3))
        osb_pool = ctx.enter_context(tc.tile_pool(name="osb", bufs=2))
        nrm_pool = ctx.enter_context(tc.tile_pool(name="nrm", bufs=4))
        pp_big = ctx.enter_context(tc.tile_pool(name="pp_big", bufs=2, space="PSUM"))
        pp_mm = ctx.enter_context(tc.tile_pool(name="pp_mm", bufs=2, space="PSUM"))
        pp_kt = ctx.enter_context(tc.tile_pool(name="pp_kt", bufs=2, space="PSUM"))
        pp_cs = ctx.enter_context(tc.tile_pool(name="pp_cs", bufs=1, space="PSUM"))
        pp_ws = ctx.enter_context(tc.tile_pool(name="pp_ws", bufs=1, space="PSUM"))

        # ---- PE warmup: keep the tensor engine busy (and ramping through its
        # p-states) while the input DMA streams in. No consumers.
        wt = singles.tile([128, 512], F16, name="warm")
        nc.vector.memset(wt, 0.125)
        for wi in range(N_WARMUP):
            ps_w = pp_big.tile([128, 512], F32, tag="big", name=f"warm{wi}")
            nc.tensor.matmul(ps_w, wt[:, 0:128], wt, start=True, stop=True)

        # ---- input DMA: critical path on the sync ring (wqkf, then xt half 0,
        # wvt, xt half 1); constants on the scalar ring. Both are HWDGE.
        xt_s = singles.tile([128, 4, T], F16)
        xt_r = xt.rearrange("(kk p) t -> p kk t", p=128)
        wqkf_s = singles.tile([128, 4, 512], F16)
        wqkf_r = wqkf.rearrange("(kk p) c -> p kk c", p=128)
        wvt_s = singles.tile([128, 4, 128], F16)
        nc.sync.dma_start(out=wqkf_s, in_=wqkf_r)
        nc.sync.dma_start(out=xt_s[:, :, 0:512], in_=xt_r[:, :, 0:512])
        nc.sync.dma_start(out=wvt_s, in_=wvt.rearrange("(kk p) c -> p kk c", p=128))
        nc.sync.dma_start(out=xt_s[:, :, 512:1024], in_=xt_r[:, :, 512:1024])

        bias_s = singles.tile([128, 5], F32, name="bias_s")
        nc.scalar.dma_start(out=bias_s, in_=biasblob[:, :])
        cs_s = singles.tile([128, T], F16)
        nc.scalar.dma_start(out=cs_s, in_=csrep[:, :])
        ident = singles.tile([128, 128], F16)
        nc.scalar.dma_start(out=ident, in_=identin[:, :])
        m0_s = singles.tile([S, SC], F16)
        nc.scalar.dma_start(out=m0_s, in_=m0in[:, :])
        w2h = []
        for h in range(2):
            t_ = singles.tile([D, E], F16, name=f"w2h{h}")
            nc.scalar.dma_start(out=t_, in_=w2[h * D:(h + 1) * D, :])
            w2h.append(t_)

        eps_t = singles.tile([1, 1], F32, name="eps_t")
        nc.vector.memset(eps_t, EPS)
        onesz_col = singles.tile([128, 2], F16, name="onesz_col")
        nc.vector.memset(onesz_col[:, 0:1], 1.0)
        nc.vector.memset(onesz_col[:, 1:2], 0.0)
        ones_row = singles.tile([1, D], F16, name="ones_row")
        nc.vector.memset(ones_row, 1.0)

        # per-head stacked feature tiles [cos;sin] x t
        qfT = [singles.tile([128, T], F16, name=f"qfT{h}") for h in range(2)]
        kfT = [singles.tile([128, T], F16, name=f"kfT{h}") for h in range(2)]
        vT = singles.tile([128, T], F16, name="vT")
        state = [singles.tile([128, D + 2], F16, name=f"state{h}") for h in range(2)]
        # persistent V' ring: [head][stripe], ones/pad cols written once
        vp_ring = [[singles.tile([S, D + 2], F16, name=f"vpr{h}_{ci}")
                    for ci in range(2)] for h in range(2)]
        for h in range(2):
            for ci in range(2):
                nc.scalar.copy(vp_ring[h][ci][:, D:D + 2], onesz_col)

        # ---- q/k/v features, first t-half fully before the second --------
        # block bi: 0=qf_h0, 1=qf_h1, 2=kf_h0, 3=kf_h1
        for th in range(2):
            tslh = slice(th * 512, (th + 1) * 512)
            for bi, dst in ((0, qfT[0]), (1, qfT[1]), (2, kfT[0]), (3, kfT[1])):
                ps = pp_big.tile([128, 512], F32, tag="big", name=f"psB{bi}_{th}")
                for kk in range(4):
                    nc.tensor.matmul(
                        ps,
                        wqkf_s[:, kk, bi * 128:(bi + 1) * 128],
                        xt_s[:, kk, tslh],
                        start=(kk == 0),
                        stop=(kk == 3),
                    )
                nc.scalar.activation(
                    out=dst[:, tslh],
                    in_=ps,
                    func=mybir.ActivationFunctionType.Relu,
                    bias=bias_s[:, bi:bi + 1],
                    scale=1.0,
                )
                nc.vector.tensor_mul(dst[:, tslh], dst[:, tslh], cs_s[:, tslh])
            ps = pp_big.tile([128, 512], F32, tag="big", name=f"psV{th}")
            for kk in range(4):
                nc.tensor.matmul(
                    ps,
                    wvt_s[:, kk, :],
                    xt_s[:, kk, tslh],
                    start=(kk == 0),
                    stop=(kk == 3),
                )
            nc.scalar.activation(
                out=vT[:, tslh],
                in_=ps,
                func=mybir.ActivationFunctionType.Identity,
                bias=bias_s[:, 4:5],
                scale=1.0,
            )

        # ---- attention, 256-wide query super-chunks ----------------------
        for sc in range(NSC):
            t0 = sc * SC
            band = slice(t0, t0 + SC)
            sub = [slice(t0, t0 + S), slice(t0 + S, t0 + 2 * S)]

            # stripe transposes: kfT/vT [*, t] -> [t, *] per 128-stripe
            kfeat = [[None, None], [None, None]]  # [ci][h]
            vp = [[None, None], [None, None]]     # [ci][h]
            kt_tiles = []
            for ci in range(2):
                ps_kt = pp_kt.tile([128, 384], F16, tag="kt", name=f"pskt{sc}_{ci}")
                kt_tiles.append(ps_kt)
                for h in range(2):
                    kfeat[ci][h] = kf_pool.tile(
                        [S, 128], F16, tag=f"kf{h}", name=f"kfeat{sc}_{ci}_{h}")
                    nc.tensor.transpose(
                        ps_kt[:, h * 128:(h + 1) * 128], kfT[h][:, sub[ci]], ident)
                nc.vector.tensor_copy(kfeat[ci][0], ps_kt[:, 0:128])
                nc.scalar.copy(kfeat[ci][1], ps_kt[:, 128:256])
                nc.tensor.transpose(ps_kt[:, 256:384], vT[:, sub[ci]], ident)
                for h in range(2):
                    vp[ci][h] = vp_ring[h][ci]
                nc.vector.tensor_copy(vp[ci][0][:, 0:D], ps_kt[:, 256:256 + D])
                nc.scalar.copy(vp[ci][1][:, 0:D], ps_kt[:, 256 + D:256 + 2 * D])

            psc_bank = pp_cs.tile([D + 2, 512], F32, tag="psc", name=f"pscb{sc}")
            ws_bank = pp_ws.tile([128, 512], F32, tag="ws", name=f"wsb{sc}")
            psc = [psc_bank[:, 0:256], psc_bank[:, 256:512]]
            pss = [ws_bank[:, 0:66], ws_bank[:, 66:132]]
            rh = [None, None]
            for h in range(2):
                # stripe 0 scores the whole band; stripe 1 only its own half
                mm_bank = pp_mm.tile([S, 384], F32, tag="mm", name=f"psa{sc}_{h}")
                ps_a0 = mm_bank[:, 0:256]
                ps_a1 = mm_bank[:, 256:384]
                nc.tensor.matmul(ps_a0, kfT[h][:, sub[0]], qfT[h][:, band],
                                 start=True, stop=True)
                atm0 = atm_pool.tile([S, SC], F16, tag="atm", name=f"atm{sc}_0_{h}")
                nc.vector.tensor_mul(atm0, ps_a0, m0_s)
                nc.tensor.matmul(ps_a1, kfT[h][:, sub[1]], qfT[h][:, sub[1]],
                                 start=True, stop=True)
                atm1 = atm_pool.tile([S, S], F16, tag="atm1", name=f"atm{sc}_1_{h}")
                nc.vector.tensor_mul(atm1, ps_a1, m0_s[:, 0:S])

                # ctx^T (+norm row 64) = prefix-state inter + two stripe intras
                ps_c = psc[h]
                if sc > 0:
                    nc.tensor.matmul(ps_c, state[h], qfT[h][:, band], start=True, stop=False)
                    nc.tensor.matmul(ps_c, vp[0][h], atm0, start=False, stop=False)
                    nc.tensor.matmul(ps_c[:, S:SC], vp[1][h], atm1, start=False, stop=True)
                else:
                    nc.tensor.matmul(ps_c, vp[0][h], atm0, start=True, stop=False)
                    nc.tensor.matmul(ps_c[:, S:SC], vp[1][h], atm1, start=False, stop=True)

                # state += Kf^T V' over both stripes
                ps_s = pss[h]
                nc.tensor.matmul(ps_s, kfeat[0][h], vp[0][h], start=True, stop=False)
                nc.tensor.matmul(ps_s, kfeat[1][h], vp[1][h], start=False, stop=True)
                if sc == 0:
                    nc.vector.tensor_copy(state[h], ps_s)
                else:
                    nc.vector.tensor_add(state[h], state[h], ps_s)

                # norm row (+eps) -> reciprocal, per head
                nrow = nrm_pool.tile([1, SC], F16, tag=f"nrow{h}", name=f"nrow{sc}_{h}")
                nc.scalar.activation(out=nrow, in_=ps_c[D:D + 1, :],
                                     func=mybir.ActivationFunctionType.Identity,
                                     bias=eps_t[0:1, 0:1], scale=1.0)
                rh[h] = nrm_pool.tile([1, SC], F16, tag=f"rh{h}", name=f"rh{sc}_{h}")
                with nc.allow_low_precision(reason="f16 datapath; 2e-2 gate"):
                    nc.vector.reciprocal(rh[h], nrow)

            # replicate each head's 1/norm row onto 64 partitions via a
            # rank-1 matmul (ones-row stationary), normalize ctx, then let
            # PSUM accumulation combine the two heads' output projections
            ctxn = [None, None]
            for h in range(2):
                ps_rep = pp_mm.tile([D, SC], F32, tag="mm",
                                    name=f"psrep{sc}_{h}")
                nc.tensor.matmul(ps_rep, ones_row, rh[h], start=True, stop=True)
                rep = nrm_pool.tile([D, SC], F16, tag=f"rep{h}",
                                    name=f"rep{sc}_{h}")
                nc.scalar.copy(rep, ps_rep)
                ctxn[h] = nrm_pool.tile([D, SC], F16, tag=f"ctxn{h}",
                                        name=f"ctxn{sc}_{h}")
                nc.vector.tensor_mul(ctxn[h], psc[h][0:D, :], rep)
            for ci in range(2):
                ps = pp_big.tile([128, E], F32, tag="big", name=f"pso{sc}_{ci}")
                nc.tensor.matmul(ps, ctxn[0][:, ci * S:(ci + 1) * S], w2h[0],
                                 start=True, stop=False)
                nc.tensor.matmul(ps, ctxn[1][:, ci * S:(ci + 1) * S], w2h[1],
                                 start=False, stop=True)
                o_s = osb_pool.tile([128, E], F16, tag="osb", name=f"os{sc}_{ci}")
                nc.scalar.copy(o_s, ps)
                if ci == 0:
                    nc.sync.dma_start(out=out[sub[ci], :], in_=o_s)
                else:
                    nc.scalar.dma_start(out=out[sub[ci], :], in_=o_s)

    _split_multi_waits(nc)
    return nc


_PROGRAM = None


def _get_program():
    global _PROGRAM
    if _PROGRAM is None:
        _PROGRAM = build_program()
    return _PROGRAM


def _make_in_maps(x, w_qkv, b_qkv, w_out):
    pos = np.arange(T, dtype=np.float32)
    ang = (math.pi / 2) * pos / T
    cosw = np.cos(ang).astype(np.float32)
    sinw = np.sin(ang).astype(np.float32)
    csrep = np.concatenate([
        np.broadcast_to(cosw[None, :], (D, T)),
        np.broadcast_to(sinw[None, :], (D, T)),
    ], 0).astype(np.float16)
    tri = np.triu(np.ones((S, S), np.float16))
    m0 = np.concatenate([tri, np.ones((S, S), np.float16)], 1)

    in_maps = []
    for i in range(8):
        b, g = divmod(i, 4)
        h0, h1 = 2 * g, 2 * g + 1
        wq = lambda h: w_qkv[h * D:(h + 1) * D]
        wk = lambda h: w_qkv[E + h * D:E + (h + 1) * D]
        wv = lambda h: w_qkv[2 * E + h * D:2 * E + (h + 1) * D]
        bq = lambda h: b_qkv[h * D:(h + 1) * D]
        bk = lambda h: b_qkv[E + h * D:E + (h + 1) * D]
        bv = lambda h: b_qkv[2 * E + h * D:2 * E + (h + 1) * D]
        hcols = np.r_[h0 * D:(h0 + 1) * D, h1 * D:(h1 + 1) * D]
        wqkf = np.concatenate([
            wq(h0), wq(h0), wq(h1), wq(h1), wk(h0), wk(h0), wk(h1), wk(h1)
        ], 0).T
        # bias blob [128, 5]: col bi = dup'd qk bias for feature block bi,
        # col 4 = v bias (both heads stacked)
        bias_cols = [
            np.concatenate([bq(h0), bq(h0)]),
            np.concatenate([bq(h1), bq(h1)]),
            np.concatenate([bk(h0), bk(h0)]),
            np.concatenate([bk(h1), bk(h1)]),
            np.concatenate([bv(h0), bv(h1)]),
        ]
        biasblob = np.stack(bias_cols, axis=1).astype(np.float32)
        in_maps.append({
            "xt": np.ascontiguousarray(x[b].T.astype(np.float16)),
            "wqkf": np.ascontiguousarray(wqkf.astype(np.float16)),
            "wvt": np.ascontiguousarray(
                np.concatenate([wv(h0), wv(h1)], 0).T.astype(np.float16)),
            "biasblob": biasblob,
            "csrep": csrep,
            "w2": np.ascontiguousarray(w_out[:, hcols].T.astype(np.float16)),
            "identin": np.eye(128, dtype=np.float16),
            "m0in": m0,
        })
    return in_maps


def run(inputs, trace=False):
    x = np.asarray(inputs["x"], dtype=np.float32)
    w_qkv = np.asarray(inputs["w_qkv"], dtype=np.float32)
    b_qkv = np.asarray(inputs["b_qkv"], dtype=np.float32)
    w_out = np.asarray(inputs["w_out"], dtype=np.float32)
    b_out = np.asarray(inputs["b_out"], dtype=np.float32)

    nc = _get_program()
    in_maps = _make_in_maps(x, w_qkv, b_qkv, w_out)
    res = run_bass_kernel_spmd(nc, in_maps, list(range(8)), trace=trace)

    out = np.empty((B, T, E), dtype=np.float32)
    for b in range(B):
        acc = res.results[4 * b]["out"].astype(np.float32)
        for g in range(1, 4):
            acc = acc + res.results[4 * b + g]["out"].astype(np.float32)
        out[b] = acc + b_out[None, :]
    return out, res


def kernel(**inputs) -> np.ndarray:
    out, _ = run(inputs, trace=False)
    return out


# revision 12
# speedup vs baseline: 1.0130x; 1.0130x over previous
"""CosFormer causal attention — Trainium2 Bass kernel, 8 NeuronCores.

Toeplitz variant: within a chunk, cos_t*cos_j + sin_t*sin_j = cos(ang_t-ang_j)
depends only on t-j, so intra-chunk scores use RAW relu features (64-dim
contraction) with the cos factor folded into the constant causal mask. The
cos/sin feature split is only needed on the inter-chunk state path: qcos/qsin
via DVE muls, and cos_j/sin_j folded into V' via per-partition scalar scales.
This cuts the qkv feature projection from 5 to 3 blocks of 128 columns
(20480 -> 12288 PE rows).

Sharding: core i = (batch b = i//4, head-group g = i%4 covering heads 2g,2g+1).
Host sums the 4 per-core partials per batch and adds b_out.

fp16 datapath (PSUM fp32), dual-HWDGE-ring input DMA, PE warmup bridge,
PSUM-accumulated head-combine on the output projection, 1/norm replicated
per head via gpsimd partition_broadcast.

Fully self-contained: hardcodes B=2, T=1024, E=512, H=8.
"""

import math
from contextlib import ExitStack

import numpy as np

import concourse.bass as bass
import concourse.mybir as mybir
import concourse.tile as tile
from concourse.bass_utils import run_bass_kernel_spmd
from concourse.vector_clock import ScopedClock

B, T, E = 2, 1024, 512
H, D = 8, 64
S = 128            # key stripe size
SC = 256           # query super-chunk size
NSC = T // SC      # 4
F32 = mybir.dt.float32
F16 = mybir.dt.float16
EPS = 1e-6
N_WARMUP = 9


def _install_drain_patch():
    if getattr(tile.TileContext, "_drain_patch_installed", False):
        return

    def _patched(self, tick_clock, wait_clock):
        nc = self.nc
        pre = nc.sync.nop(nofuse=True)
        wait_clock.add_sem_waits(pre.ins, ScopedClock({None: tick_clock.global_clock}))
        waits = list(pre.ins.sync_info.on_wait or []) if pre.ins.sync_info else []
        if len(waits) > 1:
            pre.ins.sync_info.on_wait = waits[:1]
            for w in waits[1:]:
                n = nc.sync.nop(nofuse=True)
                if n.ins.sync_info is None:
                    n.ins.sync_info = mybir.SyncInfo(on_wait=[w], on_update=[])
                else:
                    n.ins.sync_info.on_wait = [w]
        nc.sync.drain()
        nc.all_engine_barrier()
        popped = nc._tile_sem_poison_stack.pop()
        assert popped is self._sem_poison

    tile.TileContext._drain_and_barrier = _patched
    tile.TileContext._drain_patch_installed = True


def _split_multi_waits(nc):
    ctr = [0]

    def _mk_nop(engine, wait):
        ctr[0] += 1
        return mybir.InstNoOp(
            name=f"I-waitnop{ctr[0]}",
            engine=engine,
            ins=[],
            outs=[],
            sync_info=mybir.SyncInfo(on_wait=[wait], on_update=[]),
        )

    for f in nc.m.functions:
        for bb in f.blocks:
            new_insts = []
            for inst in bb.instructions:
                si = inst.sync_info
                waits = list(si.on_wait) if si and si.on_wait else []
                if len(waits) > 1:
                    for w in waits[:-1]:
                        new_insts.append(_mk_nop(inst.engine, w))
                    si.on_wait = waits[-1:]
                new_insts.append(inst)
            bb.instructions[:] = new_insts


def build_program() -> bass.Bass:
    _install_drain_patch()
    nc = bass.Bass()

    # wqkv: raw (non-duplicated) weight cols [q_h0 q_h1 | k_h0 k_h1 | v_h0 v_h1]
    xt = nc.declare_dram_parameter("xt", [E, T], F16, isOutput=False)
    wqkv = nc.declare_dram_parameter("wqkv", [E, 384], F16, isOutput=False)
    biascs = nc.declare_dram_parameter("biascs", [128, 19], F32, isOutput=False)
    cosrep = nc.declare_dram_parameter("cosrep", [128, T], F16, isOutput=False)
    sinrep = nc.declare_dram_parameter("sinrep", [128, T], F16, isOutput=False)
    w2 = nc.declare_dram_parameter("w2", [128, E], F16, isOutput=False)
    consts16 = nc.declare_dram_parameter("consts16", [128, 384], F16, isOutput=False)
    out = nc.declare_dram_parameter("out", [T, E], F16, isOutput=True)

    with tile.TileContext(nc) as tc, ExitStack() as ctx:
        singles = ctx.enter_context(tc.tile_pool(name="singles", bufs=1))
        kf_pool = ctx.enter_context(tc.tile_pool(name="kf", bufs=4))
        atm_pool = ctx.enter_context(tc.tile_pool(name="atm", bufs=3))
        osb_pool = ctx.enter_context(tc.tile_pool(name="osb", bufs=2))
        nrm_pool = ctx.enter_context(tc.tile_pool(name="nrm", bufs=4))
        vcs_pool = ctx.enter_context(tc.tile_pool(name="vcs", bufs=4))
        pp_big = ctx.enter_context(tc.tile_pool(name="pp_big", bufs=2, space="PSUM"))
        pp_mm = ctx.enter_context(tc.tile_pool(name="pp_mm", bufs=2, space="PSUM"))
        pp_kt = ctx.enter_context(tc.tile_pool(name="pp_kt", bufs=2, space="PSUM"))
        pp_cs = ctx.enter_context(tc.tile_pool(name="pp_cs", bufs=1, space="PSUM"))
        pp_ws = ctx.enter_context(tc.tile_pool(name="pp_ws", bufs=1, space="PSUM"))

        # ---- PE warmup while input DMA streams
        wt = singles.tile([128, 512], F16, name="warm")
        nc.vector.memset(wt, 0.125)
        for wi in range(N_WARMUP):
            ps_w = pp_big.tile([128, 512], F32, tag="big", name=f"warm{wi}")
            nc.tensor.matmul(ps_w, wt[:, 0:128], wt, start=True, stop=True)

        # ---- input DMA
        xt_s = singles.tile([128, 4, T], F16)
        xt_r = xt.rearrange("(kk p) t -> p kk t", p=128)
        wqkv_s = singles.tile([128, 4, 384], F16)
        wqkv_r = wqkv.rearrange("(kk p) c -> p kk c", p=128)
        cos_s = singles.tile([128, T], F16)
        nc.sync.dma_start(out=wqkv_s, in_=wqkv_r)
        nc.sync.dma_start(out=xt_s[:, :, 0:512], in_=xt_r[:, :, 0:512])
        nc.sync.dma_start(out=xt_s[:, :, 512:1024], in_=xt_r[:, :, 512:1024])
        nc.sync.dma_start(out=cos_s, in_=cosrep[:, :])

        biascs_s = singles.tile([128, 19], F32, name="biascs_s")
        nc.scalar.dma_start(out=biascs_s, in_=biascs[:, :])
        bias_s = biascs_s[:, 0:3]
        cscol_s = biascs_s[:, 3:19]
        c16_s = singles.tile([128, 384], F16, name="c16_s")
        nc.scalar.dma_start(out=c16_s, in_=consts16[:, :])
        ident = c16_s[:, 0:128]
        m0_s = c16_s[:, 128:384]
        sin_s = singles.tile([128, T], F16)
        nc.scalar.dma_start(out=sin_s, in_=sinrep[:, :])
        w2h = []
        for h in range(2):
            t_ = singles.tile([D, E], F16, name=f"w2h{h}")
            nc.scalar.dma_start(out=t_, in_=w2[h * D:(h + 1) * D, :])
            w2h.append(t_)

        eps_t = singles.tile([1, 1], F32, name="eps_t")
        nc.vector.memset(eps_t, EPS)
        onesz_col = singles.tile([128, 2], F16, name="onesz_col")
        nc.vector.memset(onesz_col[:, 0:1], 1.0)
        nc.vector.memset(onesz_col[:, 1:2], 0.0)

        # raw per-head features [d, t] and their cos/sin-scaled versions
        qr = [singles.tile([D, T], F16, name=f"qr{h}") for h in range(2)]
        kr = [singles.tile([D, T], F16, name=f"kr{h}") for h in range(2)]
        qc = [singles.tile([D, T], F16, name=f"qc{h}") for h in range(2)]
        qs = [singles.tile([D, T], F16, name=f"qs{h}") for h in range(2)]
        vT = singles.tile([128, T], F16, name="vT")
        # state: [64 k-dims, 132] = [cos: v 0:64, norm 64, pad 65 | sin: 66:132]
        state_cs = [singles.tile([D, 132], F16, name=f"scs{h}") for h in range(2)]
        # persistent raw V' ring: [head][stripe] = [S, 66] (v, ones, zero)
        vp_ring = [[singles.tile([S, 66], F16, name=f"vpr{h}_{ci}")
                    for ci in range(2)] for h in range(2)]
        for h in range(2):
            for ci in range(2):
                nc.scalar.copy(vp_ring[h][ci][:, D:D + 2], onesz_col[:, 0:2])

        # ---- q/k/v raw features; qcos/qsin built by DVE muls ------------
        # blocks: 0 = q (both heads), 1 = k, 2 = v
        for th in range(2):
            tslh = slice(th * 512, (th + 1) * 512)
            for bi, dsts in ((0, qr), (1, kr)):
                ps = pp_big.tile([128, 512], F32, tag="big", name=f"psB{bi}_{th}")
                for kk in range(4):
                    nc.tensor.matmul(
                        ps,
                        wqkv_s[:, kk, bi * 128:(bi + 1) * 128],
                        xt_s[:, kk, tslh],
                        start=(kk == 0),
                        stop=(kk == 3),
                    )
                for h in range(2):
                    nc.scalar.activation(
                        out=dsts[h][:, tslh],
                        in_=ps[h * D:(h + 1) * D, :],
                        func=mybir.ActivationFunctionType.Relu,
                        bias=bias_s[h * D:(h + 1) * D, bi:bi + 1],
                        scale=1.0,
                    )
                if bi == 0:
                    for h in range(2):
                        nc.vector.tensor_mul(qc[h][:, tslh], qr[h][:, tslh],
                                             cos_s[0:D, tslh])
                        nc.vector.tensor_mul(qs[h][:, tslh], qr[h][:, tslh],
                                             sin_s[0:D, tslh])
            ps = pp_big.tile([128, 512], F32, tag="big", name=f"psV{th}")
            for kk in range(4):
                nc.tensor.matmul(
                    ps,
                    wqkv_s[:, kk, 256:384],
                    xt_s[:, kk, tslh],
                    start=(kk == 0),
                    stop=(kk == 3),
                )
            nc.scalar.activation(
                out=vT[:, tslh],
                in_=ps,
                func=mybir.ActivationFunctionType.Identity,
                bias=bias_s[:, 2:3],
                scale=1.0,
            )

        # ---- attention, 256-wide query super-chunks ----------------------
        for sc in range(NSC):
            t0 = sc * SC
            band = slice(t0, t0 + SC)
            sub = [slice(t0, t0 + S), slice(t0 + S, t0 + 2 * S)]

            # per stripe: transpose kr (per head) and v; build raw V' and the
            # cos/sin-scaled V' (scale = cos_j/sin_j per t-partition)
            kfeat = [[None, None], [None, None]]   # [ci][h] -> [S, 64]
            vp_cs = [[None, None], [None, None]]   # [ci][h] -> [S, 132]
            kt_tiles = []
            for ci in range(2):
                si_ = 2 * sc + ci
                ps_kt = pp_kt.tile([128, 260], F16, tag="kt", name=f"pskt{sc}_{ci}")
                kt_tiles.append(ps_kt)
                for h in range(2):
                    nc.tensor.transpose(
                        ps_kt[:, h * D:(h + 1) * D], kr[h][:, sub[ci]],
                        ident[0:D, 0:D])
                nc.tensor.transpose(ps_kt[:, 128:256], vT[:, sub[ci]], ident)
                for h in range(2):
                    kfeat[ci][h] = kf_pool.tile(
                        [S, D], F16, tag=f"kf{h}", name=f"kfeat{sc}_{ci}_{h}")
                    if h == 0:
                        nc.vector.tensor_copy(kfeat[ci][h], ps_kt[:, 0:D])
                        nc.vector.tensor_copy(vp_ring[h][ci][:, 0:D],
                                              ps_kt[:, 128:128 + D])
                    else:
                        nc.scalar.copy(kfeat[ci][h], ps_kt[:, D:2 * D])
                        nc.scalar.copy(vp_ring[h][ci][:, 0:D],
                                       ps_kt[:, 128 + D:128 + 2 * D])
                    vp_cs[ci][h] = vcs_pool.tile(
                        [S, 132], F16, tag=f"vcs{h}", name=f"vpcs{sc}_{ci}_{h}")
                    nc.scalar.activation(
                        out=vp_cs[ci][h][:, 0:66], in_=vp_ring[h][ci],
                        func=mybir.ActivationFunctionType.Copy,
                        scale=cscol_s[:, 2 * si_:2 * si_ + 1])
                    nc.scalar.activation(
                        out=vp_cs[ci][h][:, 66:132], in_=vp_ring[h][ci],
                        func=mybir.ActivationFunctionType.Copy,
                        scale=cscol_s[:, 2 * si_ + 1:2 * si_ + 2])

            psc_bank = pp_cs.tile([D + 1, 512], F32, tag="psc", name=f"pscb{sc}")
            ws_bank = pp_ws.tile([D, 512], F32, tag="ws", name=f"wsb{sc}")
            psc = [psc_bank[:, 0:256], psc_bank[:, 256:512]]
            pss = [ws_bank[:, 0:132], ws_bank[:, 132:264]]
            ps_o = [[None, None], [None, None]]   # [ci][h]
            ncol = [[None, None], [None, None]]   # [ci][h]
            for h in range(2):
                # raw scores; mask carries tri * cos(ang_t - ang_j)
                mm_bank = pp_mm.tile([S, 384], F32, tag="mm", name=f"psa{sc}_{h}")
                ps_a0 = mm_bank[:, 0:256]
                ps_a1 = mm_bank[:, 256:384]
                nc.tensor.matmul(ps_a0, kr[h][:, sub[0]], qr[h][:, band],
                                 start=True, stop=True)
                atm0 = atm_pool.tile([S, SC], F16, tag="atm", name=f"atm{sc}_0_{h}")
                nc.vector.tensor_mul(atm0, ps_a0, m0_s)
                nc.tensor.matmul(ps_a1, kr[h][:, sub[1]], qr[h][:, sub[1]],
                                 start=True, stop=True)
                atm1 = atm_pool.tile([S, S], F16, tag="atm1", name=f"atm{sc}_1_{h}")
                nc.vector.tensor_mul(atm1, ps_a1, m0_s[:, 0:S])

                # ctx^T (+norm row 64): prefix state (cos+sin) + stripe intras
                ps_c = psc[h]
                if sc > 0:
                    nc.tensor.matmul(ps_c, state_cs[h][:, 0:65], qc[h][:, band],
                                     start=True, stop=False)
                    nc.tensor.matmul(ps_c, state_cs[h][:, 66:131], qs[h][:, band],
                                     start=False, stop=False)
                    nc.tensor.matmul(ps_c, vp_ring[h][0][:, 0:65], atm0,
                                     start=False, stop=False)
                    nc.tensor.matmul(ps_c[:, S:SC], vp_ring[h][1][:, 0:65], atm1,
                                     start=False, stop=True)
                else:
                    nc.tensor.matmul(ps_c, vp_ring[h][0][:, 0:65], atm0,
                                     start=True, stop=False)
                    nc.tensor.matmul(ps_c[:, S:SC], vp_ring[h][1][:, 0:65], atm1,
                                     start=False, stop=True)

                # state_cs += Kraw^T [V'cos | V'sin] over both stripes
                ps_s = pss[h]
                nc.tensor.matmul(ps_s, kfeat[0][h], vp_cs[0][h],
                                 start=True, stop=False)
                nc.tensor.matmul(ps_s, kfeat[1][h], vp_cs[1][h],
                                 start=False, stop=True)
                if sc == 0:
                    nc.vector.tensor_copy(state_cs[h], ps_s)
                else:
                    nc.vector.tensor_add(state_cs[h], state_cs[h], ps_s)

                # norm row -> [t,1] columns (PE transpose) -> reciprocal
                nrow = nrm_pool.tile([1, SC], F16, tag=f"nrow{h}", name=f"nrow{sc}_{h}")
                nc.scalar.activation(out=nrow, in_=ps_c[D:D + 1, :],
                                     func=mybir.ActivationFunctionType.Identity,
                                     bias=eps_t[0:1, 0:1], scale=1.0)
                for ci in range(2):
                    ps_n = kt_tiles[ci][:, 256 + 2 * h:258 + 2 * h]
                    nc.tensor.transpose(ps_n, nrow[:, ci * S:(ci + 1) * S],
                                        ident[0:1, 0:2])
                    nc_t = nrm_pool.tile([S, 1], F32, tag="ncol",
                                         name=f"ncol{sc}_{ci}_{h}")
                    nc.vector.reciprocal(nc_t, ps_n[:, 0:1])
                    ncol[ci][h] = nc_t

                # unnormalized ctx -> SBUF; per-stripe per-head out-projection
                ctxu = nrm_pool.tile([D, SC], F16, tag=f"ctxu{h}",
                                     name=f"ctxu{sc}_{h}")
                nc.scalar.copy(ctxu, psc[h][0:D, :])
                for ci in range(2):
                    ps = pp_big.tile([128, E], F32, tag="big",
                                     name=f"pso{sc}_{ci}_{h}")
                    nc.tensor.matmul(ps, ctxu[:, ci * S:(ci + 1) * S], w2h[h],
                                     start=True, stop=True)
                    ps_o[ci][h] = ps

            # scale by 1/norm (per-partition) and combine heads
            for ci in range(2):
                o_s = osb_pool.tile([128, E], F16, tag="osb", name=f"os{sc}_{ci}")
                nc.scalar.activation(out=o_s, in_=ps_o[ci][0],
                                     func=mybir.ActivationFunctionType.Copy,
                                     scale=ncol[ci][0])
                nc.vector.scalar_tensor_tensor(
                    out=o_s, in0=ps_o[ci][1], scalar=ncol[ci][1], in1=o_s,
                    op0=mybir.AluOpType.mult, op1=mybir.AluOpType.add,
                )
                if ci == 0:
                    nc.sync.dma_start(out=out[sub[ci], :], in_=o_s)
                else:
                    nc.scalar.dma_start(out=out[sub[ci], :], in_=o_s)

    _split_multi_waits(nc)
    return nc


_PROGRAM = None


def _get_program():
    global _PROGRAM
    if _PROGRAM is None:
        _PROGRAM = build_program()
    return _PROGRAM


def _make_in_maps(x, w_qkv, b_qkv, w_out):
    pos = np.arange(T, dtype=np.float32)
    ang = (math.pi / 2) * pos / T
    cosw = np.cos(ang).astype(np.float32)
    sinw = np.sin(ang).astype(np.float32)
    cosrep = np.ascontiguousarray(
        np.broadcast_to(cosw[None, :], (128, T))).astype(np.float16)
    sinrep = np.ascontiguousarray(
        np.broadcast_to(sinw[None, :], (128, T))).astype(np.float16)
    # Toeplitz causal mask with the relative-angle cosine folded in
    a = np.arange(S)[:, None]
    b = np.arange(SC)[None, :]
    m0 = np.where(b >= a, np.cos((math.pi / 2) * (b - a) / T), 0.0)
    m0 = m0.astype(np.float16)
    # per-stripe cos/sin columns: stripe si covers t in [si*128, (si+1)*128)
    cscol = np.empty((S, 16), np.float32)
    for si in range(8):
        cscol[:, 2 * si] = cosw[si * S:(si + 1) * S]
        cscol[:, 2 * si + 1] = sinw[si * S:(si + 1) * S]

    in_maps = []
    for i in range(8):
        bb_, g = divmod(i, 4)
        h0, h1 = 2 * g, 2 * g + 1
        wq = lambda h: w_qkv[h * D:(h + 1) * D]
        wk = lambda h: w_qkv[E + h * D:E + (h + 1) * D]
        wv = lambda h: w_qkv[2 * E + h * D:2 * E + (h + 1) * D]
        bq = lambda h: b_qkv[h * D:(h + 1) * D]
        bk = lambda h: b_qkv[E + h * D:E + (h + 1) * D]
        bv = lambda h: b_qkv[2 * E + h * D:2 * E + (h + 1) * D]
        hcols = np.r_[h0 * D:(h0 + 1) * D, h1 * D:(h1 + 1) * D]
        wqkv = np.concatenate([
            wq(h0), wq(h1), wk(h0), wk(h1), wv(h0), wv(h1)
        ], 0).T
        bias_cols = [
            np.concatenate([bq(h0), bq(h1)]),
            np.concatenate([bk(h0), bk(h1)]),
            np.concatenate([bv(h0), bv(h1)]),
        ]
        biascs = np.concatenate(
            [np.stack(bias_cols, axis=1), cscol], axis=1).astype(np.float32)
        consts16 = np.concatenate(
            [np.eye(128, dtype=np.float16), m0], axis=1).astype(np.float16)
        in_maps.append({
            "xt": np.ascontiguousarray(x[bb_].T.astype(np.float16)),
            "wqkv": np.ascontiguousarray(wqkv.astype(np.float16)),
            "biascs": biascs,
            "cosrep": cosrep,
            "sinrep": sinrep,
            "w2": np.ascontiguousarray(w_out[:, hcols].T.astype(np.float16)),
            "consts16": consts16,
        })
    return in_maps


def run(inputs, trace=False):
    x = np.asarray(inputs["x"], dtype=np.float32)
    w_qkv = np.asarray(inputs["w_qkv"], dtype=np.float32)
    b_qkv = np.asarray(inputs["b_qkv"], dtype=np.float32)
    w_out = np.asarray(inputs["w_out"], dtype=np.float32)
    b_out = np.asarray(inputs["b_out"], dtype=np.float32)

    nc = _get_program()
    in_maps = _make_in_maps(x, w_qkv, b_qkv, w_out)
    res = run_bass_kernel_spmd(nc, in_maps, list(range(8)), trace=trace)

    out = np.empty((B, T, E), dtype=np.float32)
    for b in range(B):
        acc = res.results[4 * b]["out"].astype(np.float32)
        for g in range(1, 4):
            acc = acc + res.results[4 * b + g]["out"].astype(np.float32)
        out[b] = acc + b_out[None, :]
    return out, res


def kernel(**inputs) -> np.ndarray:
    out, _ = run(inputs, trace=False)
    return out


# revision 13
# speedup vs baseline: 1.0352x; 1.0220x over previous
"""CosFormer causal attention — Trainium2 Bass kernel, 8 NeuronCores.

Sharding: core i = (batch b = i//4, head-group g = i%4 covering heads 2g, 2g+1).
Each core computes the qkv projection for its two heads, chunked causal linear
attention (cos/sin feature channels), and a partial output projection over its
128 context channels. The host unshards by summing the 4 per-core partials of
each batch (the output projection's contraction is sharded over heads) and
adding b_out.

Key layout/perf choices:
- fp16 datapath end to end (PSUM accumulation in fp32): 1 cycle/row on the PE
  at every moving size (fp32r needs moving>=256 and pays 4x below), half the
  HBM traffic of fp32, and ~5e-4 relative rounding - far inside the 2e-2 gate.
- Input DMA split across both HWDGE rings (sync + scalar), coalesced into a
  few large transfers ordered critical-first; no SWDGE (gpsimd) traffic.
- PE warmup chain bridges the initial DMA wait so the tensor engine is past
  its DVFS ramp (0.65/1.2/2.4 GHz p-states) when the real matmuls start.
- Per-head q/k features in [feat, t] layout as one [128, T] tile (rows 0:64 =
  relu(.)*cos_t, 64:128 = relu(.)*sin_t), produced by projecting with
  duplicated weight columns (PE cost scales with the moving dim, not M).
- Attention runs in 256-wide query super-chunks: two 128-wide key stripes are
  scored against the full 256 query band and masked. A [2d, d+2] state carries
  the prefix between super-chunks.
- Normalization: norm row -> PE-transpose to a [t, 1] column -> 1-elem/lane
  reciprocal -> applied per-partition to the per-head out-projection partials.

Fully self-contained: hardcodes B=2, T=1024, E=512, H=8.
"""

import math
from contextlib import ExitStack

import numpy as np

import concourse.bass as bass
import concourse.mybir as mybir
import concourse.tile as tile
from concourse.bass_utils import run_bass_kernel_spmd
from concourse.vector_clock import ScopedClock

B, T, E = 2, 1024, 512
H, D = 8, 64
S = 128            # key stripe size
SC = 256           # query super-chunk size
NSC = T // SC      # 4
F32 = mybir.dt.float32
F16 = mybir.dt.float16
EPS = 1e-6
N_WARMUP = 9


def _install_drain_patch():
    """This walrus build rejects a Drain carrying >1 sem wait. Split the
    Tile-exit drain's waits across single-wait SP nops."""
    if getattr(tile.TileContext, "_drain_patch_installed", False):
        return

    def _patched(self, tick_clock, wait_clock):
        nc = self.nc
        pre = nc.sync.nop(nofuse=True)
        wait_clock.add_sem_waits(pre.ins, ScopedClock({None: tick_clock.global_clock}))
        waits = list(pre.ins.sync_info.on_wait or []) if pre.ins.sync_info else []
        if len(waits) > 1:
            pre.ins.sync_info.on_wait = waits[:1]
            for w in waits[1:]:
                n = nc.sync.nop(nofuse=True)
                if n.ins.sync_info is None:
                    n.ins.sync_info = mybir.SyncInfo(on_wait=[w], on_update=[])
                else:
                    n.ins.sync_info.on_wait = [w]
        nc.sync.drain()
        nc.all_engine_barrier()
        popped = nc._tile_sem_poison_stack.pop()
        assert popped is self._sem_poison

    tile.TileContext._drain_and_barrier = _patched
    tile.TileContext._drain_patch_installed = True


def _split_multi_waits(nc):
    """This walrus build only codegens ONE sync-wait command per instruction.
    Move excess waits onto same-engine NoOps inserted just before."""
    ctr = [0]

    def _mk_nop(engine, wait):
        ctr[0] += 1
        return mybir.InstNoOp(
            name=f"I-waitnop{ctr[0]}",
            engine=engine,
            ins=[],
            outs=[],
            sync_info=mybir.SyncInfo(on_wait=[wait], on_update=[]),
        )

    for f in nc.m.functions:
        for bb in f.blocks:
            new_insts = []
            for inst in bb.instructions:
                si = inst.sync_info
                waits = list(si.on_wait) if si and si.on_wait else []
                if len(waits) > 1:
                    for w in waits[:-1]:
                        new_insts.append(_mk_nop(inst.engine, w))
                    si.on_wait = waits[-1:]
                new_insts.append(inst)
            bb.instructions[:] = new_insts


def build_program() -> bass.Bass:
    _install_drain_patch()
    nc = bass.Bass()

    # wqkf: duplicated weight cols [qf_h0 | qf_h1 | kf_h0 | kf_h1], each 128 wide
    xt = nc.declare_dram_parameter("xt", [E, T], F16, isOutput=False)        # x[b].T
    wqkf = nc.declare_dram_parameter("wqkf", [E, 512], F16, isOutput=False)
    wvt = nc.declare_dram_parameter("wvt", [E, 128], F16, isOutput=False)    # [v0 v1].T
    biasblob = nc.declare_dram_parameter("biasblob", [128, 5], F32, isOutput=False)
    csrep = nc.declare_dram_parameter("csrep", [128, T], F16, isOutput=False)  # [cos;sin]
    w2 = nc.declare_dram_parameter("w2", [128, E], F16, isOutput=False)
    identin = nc.declare_dram_parameter("identin", [128, 128], F16, isOutput=False)
    m0in = nc.declare_dram_parameter("m0in", [S, SC], F16, isOutput=False)    # [tri | ones]
    out = nc.declare_dram_parameter("out", [T, E], F16, isOutput=True)

    with tile.TileContext(nc) as tc, ExitStack() as ctx:
        singles = ctx.enter_context(tc.tile_pool(name="singles", bufs=1))
        kf_pool = ctx.enter_context(tc.tile_pool(name="kf", bufs=4))
        atm_pool = ctx.enter_context(tc.tile_pool(name="atm", bufs=3))
        osb_pool = ctx.enter_context(tc.tile_pool(name="osb", bufs=2))
        nrm_pool = ctx.enter_context(tc.tile_pool(name="nrm", bufs=4))
        pp_big = ctx.enter_context(tc.tile_pool(name="pp_big", bufs=2, space="PSUM"))
        pp_mm = ctx.enter_context(tc.tile_pool(name="pp_mm", bufs=2, space="PSUM"))
        pp_kt = ctx.enter_context(tc.tile_pool(name="pp_kt", bufs=2, space="PSUM"))
        pp_cs = ctx.enter_context(tc.tile_pool(name="pp_cs", bufs=2, space="PSUM"))

        # ---- PE warmup: keep the tensor engine busy (and ramping through its
        # p-states) while the input DMA streams in. No consumers.
        wt = singles.tile([128, 512], F16, name="warm")
        nc.vector.memset(wt, 0.125)
        for wi in range(N_WARMUP):
            ps_w = pp_big.tile([128, 512], F32, tag="big", name=f"warm{wi}")
            nc.tensor.matmul(ps_w, wt[:, 0:128], wt, start=True, stop=True)

        # ---- input DMA: critical path on the sync ring (wqkf, then xt half 0,
        # wvt, xt half 1); constants on the scalar ring. Both are HWDGE.
        xt_s = singles.tile([128, 4, T], F16)
        xt_r = xt.rearrange("(kk p) t -> p kk t", p=128)
        wqkf_s = singles.tile([128, 4, 512], F16)
        wqkf_r = wqkf.rearrange("(kk p) c -> p kk c", p=128)
        wvt_s = singles.tile([128, 4, 128], F16)
        nc.sync.dma_start(out=wqkf_s, in_=wqkf_r)
        nc.sync.dma_start(out=xt_s[:, :, 0:512], in_=xt_r[:, :, 0:512])
        nc.sync.dma_start(out=wvt_s, in_=wvt.rearrange("(kk p) c -> p kk c", p=128))
        nc.sync.dma_start(out=xt_s[:, :, 512:1024], in_=xt_r[:, :, 512:1024])

        bias_s = singles.tile([128, 5], F32, name="bias_s")
        nc.scalar.dma_start(out=bias_s, in_=biasblob[:, :])
        cs_s = singles.tile([128, T], F16)
        nc.scalar.dma_start(out=cs_s, in_=csrep[:, :])
        ident = singles.tile([128, 128], F16)
        nc.scalar.dma_start(out=ident, in_=identin[:, :])
        m0_s = singles.tile([S, SC], F16)
        nc.scalar.dma_start(out=m0_s, in_=m0in[:, :])
        w2h = []
        for h in range(2):
            t_ = singles.tile([D, E], F16, name=f"w2h{h}")
            nc.scalar.dma_start(out=t_, in_=w2[h * D:(h + 1) * D, :])
            w2h.append(t_)

        eps_t = singles.tile([1, 1], F32, name="eps_t")
        nc.vector.memset(eps_t, EPS)
        onesz_col = singles.tile([128, 2], F16, name="onesz_col")
        nc.vector.memset(onesz_col[:, 0:1], 1.0)
        nc.vector.memset(onesz_col[:, 1:2], 0.0)

        # per-head stacked feature tiles [cos;sin] x t
        qfT = [singles.tile([128, T], F16, name=f"qfT{h}") for h in range(2)]
        kfT = [singles.tile([128, T], F16, name=f"kfT{h}") for h in range(2)]
        vT = singles.tile([128, T], F16, name="vT")
        state = [singles.tile([128, D + 2], F16, name=f"state{h}") for h in range(2)]
        # persistent V' ring: [head][stripe], ones/pad cols written once
        vp_ring = [[singles.tile([S, D + 2], F16, name=f"vpr{h}_{ci}")
                    for ci in range(2)] for h in range(2)]
        for h in range(2):
            for ci in range(2):
                nc.scalar.copy(vp_ring[h][ci][:, D:D + 2], onesz_col)

        # ---- q/k/v features, first t-half fully before the second --------
        # block bi: 0=qf_h0, 1=qf_h1, 2=kf_h0, 3=kf_h1
        for th in range(2):
            tslh = slice(th * 512, (th + 1) * 512)
            for bi, dst in ((0, qfT[0]), (1, qfT[1]), (2, kfT[0]), (3, kfT[1])):
                ps = pp_big.tile([128, 512], F32, tag="big", name=f"psB{bi}_{th}")
                for kk in range(4):
                    nc.tensor.matmul(
                        ps,
                        wqkf_s[:, kk, bi * 128:(bi + 1) * 128],
                        xt_s[:, kk, tslh],
                        start=(kk == 0),
                        stop=(kk == 3),
                    )
                nc.scalar.activation(
                    out=dst[:, tslh],
                    in_=ps,
                    func=mybir.ActivationFunctionType.Relu,
                    bias=bias_s[:, bi:bi + 1],
                    scale=1.0,
                )
                nc.vector.tensor_mul(dst[:, tslh], dst[:, tslh], cs_s[:, tslh])
            ps = pp_big.tile([128, 512], F32, tag="big", name=f"psV{th}")
            for kk in range(4):
                nc.tensor.matmul(
                    ps,
                    wvt_s[:, kk, :],
                    xt_s[:, kk, tslh],
                    start=(kk == 0),
                    stop=(kk == 3),
                )
            nc.scalar.activation(
                out=vT[:, tslh],
                in_=ps,
                func=mybir.ActivationFunctionType.Identity,
                bias=bias_s[:, 4:5],
                scale=1.0,
            )

        # ---- attention, 256-wide query super-chunks ----------------------
        for sc in range(NSC):
            t0 = sc * SC
            band = slice(t0, t0 + SC)
            sub = [slice(t0, t0 + S), slice(t0 + S, t0 + 2 * S)]

            # stripe transposes: kfT/vT [*, t] -> [t, *] per 128-stripe
            kfeat = [[None, None], [None, None]]  # [ci][h]
            vp = [[None, None], [None, None]]     # [ci][h]
            kt_tiles = []
            for ci in range(2):
                ps_kt = pp_kt.tile([128, 392], F16, tag="kt", name=f"pskt{sc}_{ci}")
                kt_tiles.append(ps_kt)
                for h in range(2):
                    kfeat[ci][h] = kf_pool.tile(
                        [S, 128], F16, tag=f"kf{h}", name=f"kfeat{sc}_{ci}_{h}")
                    nc.tensor.transpose(
                        ps_kt[:, h * 128:(h + 1) * 128], kfT[h][:, sub[ci]], ident)
                nc.vector.tensor_copy(kfeat[ci][0], ps_kt[:, 0:128])
                nc.scalar.copy(kfeat[ci][1], ps_kt[:, 128:256])
                nc.tensor.transpose(ps_kt[:, 256:384], vT[:, sub[ci]], ident)
                for h in range(2):
                    vp[ci][h] = vp_ring[h][ci]
                nc.vector.tensor_copy(vp[ci][0][:, 0:D], ps_kt[:, 256:256 + D])
                nc.scalar.copy(vp[ci][1][:, 0:D], ps_kt[:, 256 + D:256 + 2 * D])

            ps_o = [[None, None], [None, None]]   # [ci][h]
            ncol = [[None, None], [None, None]]   # [ci][h]
            for h in range(2):
                # stripe 0 scores the whole band; stripe 1 only its own half
                ps_a0 = pp_mm.tile([S, SC], F32, tag="mm", name=f"psa{sc}_0_{h}")
                nc.tensor.matmul(ps_a0, kfT[h][:, sub[0]], qfT[h][:, band],
                                 start=True, stop=True)
                atm0 = atm_pool.tile([S, SC], F16, tag="atm", name=f"atm{sc}_0_{h}")
                nc.vector.tensor_mul(atm0, ps_a0, m0_s)
                ps_a1 = pp_mm.tile([S, S], F32, tag="mm", name=f"psa{sc}_1_{h}")
                nc.tensor.matmul(ps_a1, kfT[h][:, sub[1]], qfT[h][:, sub[1]],
                                 start=True, stop=True)
                atm1 = atm_pool.tile([S, S], F16, tag="atm1", name=f"atm{sc}_1_{h}")
                nc.vector.tensor_mul(atm1, ps_a1, m0_s[:, 0:S])

                # ctx^T (+norm row 64) = prefix-state inter + two stripe intras
                ps_c = pp_cs.tile([D + 2, SC], F32, tag="cs", name=f"psc{sc}_{h}")
                if sc > 0:
                    nc.tensor.matmul(ps_c, state[h], qfT[h][:, band], start=True, stop=False)
                    nc.tensor.matmul(ps_c, vp[0][h], atm0, start=False, stop=False)
                    nc.tensor.matmul(ps_c[:, S:SC], vp[1][h], atm1, start=False, stop=True)
                else:
                    nc.tensor.matmul(ps_c, vp[0][h], atm0, start=True, stop=False)
                    nc.tensor.matmul(ps_c[:, S:SC], vp[1][h], atm1, start=False, stop=True)

                # state += Kf^T V' over both stripes
                ps_s = pp_cs.tile([128, D + 2], F32, tag="cs", name=f"pss{sc}_{h}")
                nc.tensor.matmul(ps_s, kfeat[0][h], vp[0][h], start=True, stop=False)
                nc.tensor.matmul(ps_s, kfeat[1][h], vp[1][h], start=False, stop=True)
                if sc == 0:
                    nc.vector.tensor_copy(state[h], ps_s)
                else:
                    nc.vector.tensor_add(state[h], state[h], ps_s)

                # norm row -> [t,1] columns (PE transpose) -> reciprocal
                nrow = nrm_pool.tile([1, SC], F16, tag="nrow", name=f"nrow{sc}_{h}")
                nc.scalar.activation(out=nrow, in_=ps_c[D:D + 1, :],
                                     func=mybir.ActivationFunctionType.Identity,
                                     bias=eps_t[0:1, 0:1], scale=1.0)
                for ci in range(2):
                    ps_n = kt_tiles[ci][:, 384 + 2 * h:386 + 2 * h]
                    nc.tensor.transpose(ps_n, nrow[:, ci * S:(ci + 1) * S], ident[0:1, 0:2])
                    nc_t = nrm_pool.tile([S, 1], F32, tag="ncol", name=f"ncol{sc}_{ci}_{h}")
                    nc.vector.reciprocal(nc_t, ps_n[:, 0:1])
                    ncol[ci][h] = nc_t

                # unnormalized ctx -> SBUF; per-stripe per-head out-projection
                ctxu = nrm_pool.tile([D, SC], F16, tag="ctxu", name=f"ctxu{sc}_{h}")
                nc.scalar.copy(ctxu, ps_c[0:D, :])
                for ci in range(2):
                    ps = pp_big.tile([128, E], F32, tag="big", name=f"pso{sc}_{ci}_{h}")
                    nc.tensor.matmul(ps, ctxu[:, ci * S:(ci + 1) * S], w2h[h],
                                     start=True, stop=True)
                    ps_o[ci][h] = ps

            # scale by 1/norm (per-partition) and combine heads
            for ci in range(2):
                o_s = osb_pool.tile([128, E], F16, tag="osb", name=f"os{sc}_{ci}")
                nc.scalar.activation(out=o_s, in_=ps_o[ci][0],
                                     func=mybir.ActivationFunctionType.Copy,
                                     scale=ncol[ci][0])
                nc.vector.scalar_tensor_tensor(
                    out=o_s, in0=ps_o[ci][1], scalar=ncol[ci][1], in1=o_s,
                    op0=mybir.AluOpType.mult, op1=mybir.AluOpType.add,
                )
                nc.sync.dma_start(out=out[sub[ci], :], in_=o_s)

    _split_multi_waits(nc)
    return nc


_PROGRAM = None


def _get_program():
    global _PROGRAM
    if _PROGRAM is None:
        _PROGRAM = build_program()
    return _PROGRAM


def _make_in_maps(x, w_qkv, b_qkv, w_out):
    pos = np.arange(T, dtype=np.float32)
    ang = (math.pi / 2) * pos / T
    cosw = np.cos(ang).astype(np.float32)
    sinw = np.sin(ang).astype(np.float32)
    csrep = np.concatenate([
        np.broadcast_to(cosw[None, :], (D, T)),
        np.broadcast_to(sinw[None, :], (D, T)),
    ], 0).astype(np.float16)
    tri = np.triu(np.ones((S, S), np.float16))
    m0 = np.concatenate([tri, np.ones((S, S), np.float16)], 1)

    in_maps = []
    for i in range(8):
        b, g = divmod(i, 4)
        h0, h1 = 2 * g, 2 * g + 1
        wq = lambda h: w_qkv[h * D:(h + 1) * D]
        wk = lambda h: w_qkv[E + h * D:E + (h + 1) * D]
        wv = lambda h: w_qkv[2 * E + h * D:2 * E + (h + 1) * D]
        bq = lambda h: b_qkv[h * D:(h + 1) * D]
        bk = lambda h: b_qkv[E + h * D:E + (h + 1) * D]
        bv = lambda h: b_qkv[2 * E + h * D:2 * E + (h + 1) * D]
        hcols = np.r_[h0 * D:(h0 + 1) * D, h1 * D:(h1 + 1) * D]
        wqkf = np.concatenate([
            wq(h0), wq(h0), wq(h1), wq(h1), wk(h0), wk(h0), wk(h1), wk(h1)
        ], 0).T
        # bias blob [128, 5]: col bi = dup'd qk bias for feature block bi,
        # col 4 = v bias (both heads stacked)
        bias_cols = [
            np.concatenate([bq(h0), bq(h0)]),
            np.concatenate([bq(h1), bq(h1)]),
            np.concatenate([bk(h0), bk(h0)]),
            np.concatenate([bk(h1), bk(h1)]),
            np.concatenate([bv(h0), bv(h1)]),
        ]
        biasblob = np.stack(bias_cols, axis=1).astype(np.float32)
        in_maps.append({
            "xt": np.ascontiguousarray(x[b].T.astype(np.float16)),
            "wqkf": np.ascontiguousarray(wqkf.astype(np.float16)),
            "wvt": np.ascontiguousarray(
                np.concatenate([wv(h0), wv(h1)], 0).T.astype(np.float16)),
            "biasblob": biasblob,
            "csrep": csrep,
            "w2": np.ascontiguousarray(w_out[:, hcols].T.astype(np.float16)),
            "identin": np.eye(128, dtype=np.float16),
            "m0in": m0,
        })
    return in_maps


def run(inputs, trace=False):
    x = np.asarray(inputs["x"], dtype=np.float32)
    w_qkv = np.asarray(inputs["w_qkv"], dtype=np.float32)
    b_qkv = np.asarray(inputs["b_qkv"], dtype=np.float32)
    w_out = np.asarray(inputs["w_out"], dtype=np.float32)
    b_out = np.asarray(inputs["b_out"], dtype=np.float32)

    nc = _get_program()
    in_maps = _make_in_maps(x, w_qkv, b_qkv, w_out)
    res = run_bass_kernel_spmd(nc, in_maps, list(range(8)), trace=trace)

    out = np.empty((B, T, E), dtype=np.float32)
    for b in range(B):
        acc = res.results[4 * b]["out"].astype(np.float32)
        for g in range(1, 4):
            acc = acc + res.results[4 * b + g]["out"].astype(np.float32)
        out[b] = acc + b_out[None, :]
    return out, res


def kernel(**inputs) -> np.ndarray:
    out, _ = run(inputs, trace=False)
    return out


# revision 14
# speedup vs baseline: 1.1115x; 1.0736x over previous
"""CosFormer causal attention — Trainium2 Bass kernel, 8 NeuronCores.

Sharding: core i = (batch b = i//4, head-group g = i%4 covering heads 2g, 2g+1).
Each core computes the qkv projection for its two heads, chunked causal linear
attention (cos/sin feature channels), and a partial output projection over its
128 context channels. The host unshards by summing the 4 per-core partials of
each batch (the output projection's contraction is sharded over heads) and
adding b_out.

Key layout/perf choices:
- fp16 datapath end to end (PSUM accumulation in fp32): 1 cycle/row on the PE
  at every moving size (fp32r needs moving>=256 and pays 4x below), half the
  HBM traffic of fp32, and ~5e-4 relative rounding - far inside the 2e-2 gate.
- Input DMA split across both HWDGE rings (sync + scalar), coalesced into a
  few large transfers ordered critical-first; no SWDGE (gpsimd) traffic.
- PE warmup chain bridges the initial DMA wait so the tensor engine is past
  its DVFS ramp (0.65/1.2/2.4 GHz p-states) when the real matmuls start.
- Per-head q/k features in [feat, t] layout as one [128, T] tile (rows 0:64 =
  relu(.)*cos_t, 64:128 = relu(.)*sin_t), produced by projecting with
  duplicated weight columns (PE cost scales with the moving dim, not M).
- Attention runs in 256-wide query super-chunks: two 128-wide key stripes are
  scored against the full 256 query band and masked. A [2d, d+2] state carries
  the prefix between super-chunks.
- Normalization: norm row -> PE-transpose to a [t, 1] column -> 1-elem/lane
  reciprocal -> applied per-partition to the per-head out-projection partials.

Fully self-contained: hardcodes B=2, T=1024, E=512, H=8.
"""

import math
from contextlib import ExitStack

import numpy as np

import concourse.bass as bass
import concourse.mybir as mybir
import concourse.tile as tile
from concourse.bass_utils import run_bass_kernel_spmd
from concourse.vector_clock import ScopedClock

B, T, E = 2, 1024, 512
H, D = 8, 64
S = 128            # key stripe size
SC = 256           # query super-chunk size
NSC = T // SC      # 4
F32 = mybir.dt.float32
F16 = mybir.dt.float16
EPS = 1e-6
N_WARMUP = 4


def _install_drain_patch():
    """This walrus build rejects a Drain carrying >1 sem wait. Split the
    Tile-exit drain's waits across single-wait SP nops."""
    if getattr(tile.TileContext, "_drain_patch_installed", False):
        return

    def _patched(self, tick_clock, wait_clock):
        nc = self.nc
        pre = nc.sync.nop(nofuse=True)
        wait_clock.add_sem_waits(pre.ins, ScopedClock({None: tick_clock.global_clock}))
        waits = list(pre.ins.sync_info.on_wait or []) if pre.ins.sync_info else []
        if len(waits) > 1:
            pre.ins.sync_info.on_wait = waits[:1]
            for w in waits[1:]:
                n = nc.sync.nop(nofuse=True)
                if n.ins.sync_info is None:
                    n.ins.sync_info = mybir.SyncInfo(on_wait=[w], on_update=[])
                else:
                    n.ins.sync_info.on_wait = [w]
        nc.sync.drain()
        nc.all_engine_barrier()
        popped = nc._tile_sem_poison_stack.pop()
        assert popped is self._sem_poison

    tile.TileContext._drain_and_barrier = _patched
    tile.TileContext._drain_patch_installed = True


def _split_multi_waits(nc):
    """This walrus build only codegens ONE sync-wait command per instruction.
    Move excess waits onto same-engine NoOps inserted just before."""
    ctr = [0]

    def _mk_nop(engine, wait):
        ctr[0] += 1
        return mybir.InstNoOp(
            name=f"I-waitnop{ctr[0]}",
            engine=engine,
            ins=[],
            outs=[],
            sync_info=mybir.SyncInfo(on_wait=[wait], on_update=[]),
        )

    for f in nc.m.functions:
        for bb in f.blocks:
            new_insts = []
            for inst in bb.instructions:
                si = inst.sync_info
                waits = list(si.on_wait) if si and si.on_wait else []
                if len(waits) > 1:
                    for w in waits[:-1]:
                        new_insts.append(_mk_nop(inst.engine, w))
                    si.on_wait = waits[-1:]
                new_insts.append(inst)
            bb.instructions[:] = new_insts


def build_program() -> bass.Bass:
    _install_drain_patch()
    nc = bass.Bass()

    # wqkf: duplicated weight cols [qf_h0 | qf_h1 | kf_h0 | kf_h1], each 128 wide
    xt = nc.declare_dram_parameter("xt", [E, T], F16, isOutput=False)        # x[b].T
    wqkf = nc.declare_dram_parameter("wqkf", [E, 512], F16, isOutput=False)
    wvt = nc.declare_dram_parameter("wvt", [E, 128], F16, isOutput=False)    # [v0 v1].T
    biasblob = nc.declare_dram_parameter("biasblob", [128, 5], F32, isOutput=False)
    csrep = nc.declare_dram_parameter("csrep", [128, T], F16, isOutput=False)  # [cos;sin]
    w2 = nc.declare_dram_parameter("w2", [128, E], F16, isOutput=False)
    identin = nc.declare_dram_parameter("identin", [128, 128], F16, isOutput=False)
    m0in = nc.declare_dram_parameter("m0in", [S, SC], F16, isOutput=False)    # [tri | ones]
    out = nc.declare_dram_parameter("out", [T, E], F16, isOutput=True)

    with tile.TileContext(nc) as tc, ExitStack() as ctx:
        singles = ctx.enter_context(tc.tile_pool(name="singles", bufs=1))
        kf_pool = ctx.enter_context(tc.tile_pool(name="kf", bufs=4))
        atm_pool = ctx.enter_context(tc.tile_pool(name="atm", bufs=3))
        osb_pool = ctx.enter_context(tc.tile_pool(name="osb", bufs=2))
        nrm_pool = ctx.enter_context(tc.tile_pool(name="nrm", bufs=4))
        pp_big = ctx.enter_context(tc.tile_pool(name="pp_big", bufs=2, space="PSUM"))
        pp_mm = ctx.enter_context(tc.tile_pool(name="pp_mm", bufs=2, space="PSUM"))
        pp_kt = ctx.enter_context(tc.tile_pool(name="pp_kt", bufs=2, space="PSUM"))
        pp_cs = ctx.enter_context(tc.tile_pool(name="pp_cs", bufs=2, space="PSUM"))

        # ---- PE warmup: keep the tensor engine busy (and ramping through its
        # p-states) while the input DMA streams in. No consumers.
        wt = singles.tile([128, 512], F16, name="warm")
        nc.gpsimd.memset(wt, 0.125)
        for wi in range(N_WARMUP):
            ps_w = pp_big.tile([128, 512], F32, tag="big", name=f"warm{wi}")
            nc.tensor.matmul(ps_w, wt[:, 0:128], wt, start=True, stop=True)

        # ---- input DMA: critical path on the sync ring (wqkf, then xt half 0,
        # wvt, xt half 1); constants on the scalar ring. Both are HWDGE.
        xt_s = singles.tile([128, 4, T], F16)
        xt_r = xt.rearrange("(kk p) t -> p kk t", p=128)
        wqkf_s = singles.tile([128, 4, 512], F16)
        wqkf_r = wqkf.rearrange("(kk p) c -> p kk c", p=128)
        wvt_s = singles.tile([128, 4, 128], F16)
        nc.sync.dma_start(out=wqkf_s, in_=wqkf_r)
        nc.sync.dma_start(out=wvt_s, in_=wvt.rearrange("(kk p) c -> p kk c", p=128))
        nc.sync.dma_start(out=xt_s[:, :, 512:1024], in_=xt_r[:, :, 512:1024])

        nc.scalar.dma_start(out=xt_s[:, :, 0:512], in_=xt_r[:, :, 0:512])
        bias_s = singles.tile([128, 5], F32, name="bias_s")
        nc.scalar.dma_start(out=bias_s, in_=biasblob[:, :])
        cs_s = singles.tile([128, T], F16)
        nc.scalar.dma_start(out=cs_s, in_=csrep[:, :])
        ident = singles.tile([128, 128], F16)
        nc.scalar.dma_start(out=ident, in_=identin[:, :])
        m0_s = singles.tile([S, SC], F16)
        nc.scalar.dma_start(out=m0_s, in_=m0in[:, :])
        w2h = []
        for h in range(2):
            t_ = singles.tile([D, E], F16, name=f"w2h{h}")
            nc.scalar.dma_start(out=t_, in_=w2[h * D:(h + 1) * D, :])
            w2h.append(t_)

        eps_t = singles.tile([1, 1], F32, name="eps_t")
        nc.vector.memset(eps_t, EPS)
        onesz_col = singles.tile([128, 2], F16, name="onesz_col")
        nc.vector.memset(onesz_col[:, 0:1], 1.0)
        nc.vector.memset(onesz_col[:, 1:2], 0.0)

        # per-head stacked feature tiles [cos;sin] x t
        qfT = [singles.tile([128, T], F16, name=f"qfT{h}") for h in range(2)]
        kfT = [singles.tile([128, T], F16, name=f"kfT{h}") for h in range(2)]
        vT = singles.tile([128, T], F16, name="vT")
        state = [singles.tile([128, D + 2], F16, name=f"state{h}") for h in range(2)]
        # persistent V' ring: [head][stripe], ones/pad cols written once
        vp_ring = [[singles.tile([S, D + 2], F16, name=f"vpr{h}_{ci}")
                    for ci in range(2)] for h in range(2)]
        for h in range(2):
            for ci in range(2):
                nc.scalar.copy(vp_ring[h][ci][:, D:D + 2], onesz_col)

        # ---- q/k/v features, first t-half fully before the second --------
        # block bi: 0=qf_h0, 1=qf_h1, 2=kf_h0, 3=kf_h1
        for th in range(2):
            tslh = slice(th * 512, (th + 1) * 512)
            for bi, dst in ((0, qfT[0]), (1, qfT[1]), (2, kfT[0]), (3, kfT[1])):
                ps = pp_big.tile([128, 512], F32, tag="big", name=f"psB{bi}_{th}")
                for kk in range(4):
                    nc.tensor.matmul(
                        ps,
                        wqkf_s[:, kk, bi * 128:(bi + 1) * 128],
                        xt_s[:, kk, tslh],
                        start=(kk == 0),
                        stop=(kk == 3),
                    )
                nc.scalar.activation(
                    out=dst[:, tslh],
                    in_=ps,
                    func=mybir.ActivationFunctionType.Relu,
                    bias=bias_s[:, bi:bi + 1],
                    scale=1.0,
                )
                nc.vector.tensor_mul(dst[:, tslh], dst[:, tslh], cs_s[:, tslh])
            ps = pp_big.tile([128, 512], F32, tag="big", name=f"psV{th}")
            for kk in range(4):
                nc.tensor.matmul(
                    ps,
                    wvt_s[:, kk, :],
                    xt_s[:, kk, tslh],
                    start=(kk == 0),
                    stop=(kk == 3),
                )
            nc.scalar.activation(
                out=vT[:, tslh],
                in_=ps,
                func=mybir.ActivationFunctionType.Identity,
                bias=bias_s[:, 4:5],
                scale=1.0,
            )

        # ---- attention, 256-wide query super-chunks ----------------------
        for sc in range(NSC):
            t0 = sc * SC
            band = slice(t0, t0 + SC)
            sub = [slice(t0, t0 + S), slice(t0 + S, t0 + 2 * S)]

            # stripe transposes: kfT/vT [*, t] -> [t, *] per 128-stripe
            kfeat = [[None, None], [None, None]]  # [ci][h]
            vp = [[None, None], [None, None]]     # [ci][h]
            kt_tiles = []
            for ci in range(2):
                ps_kt = pp_kt.tile([128, 392], F16, tag="kt", name=f"pskt{sc}_{ci}")
                kt_tiles.append(ps_kt)
                for h in range(2):
                    kfeat[ci][h] = kf_pool.tile(
                        [S, 128], F16, tag=f"kf{h}", name=f"kfeat{sc}_{ci}_{h}")
                    nc.tensor.transpose(
                        ps_kt[:, h * 128:(h + 1) * 128], kfT[h][:, sub[ci]], ident)
                nc.vector.tensor_copy(kfeat[ci][0], ps_kt[:, 0:128])
                nc.scalar.copy(kfeat[ci][1], ps_kt[:, 128:256])
                nc.tensor.transpose(ps_kt[:, 256:384], vT[:, sub[ci]], ident)
                for h in range(2):
                    vp[ci][h] = vp_ring[h][ci]
                nc.vector.tensor_copy(vp[ci][0][:, 0:D], ps_kt[:, 256:256 + D])
                nc.scalar.copy(vp[ci][1][:, 0:D], ps_kt[:, 256 + D:256 + 2 * D])

            ps_o = [[None, None], [None, None]]   # [ci][h]
            ncol = [[None, None], [None, None]]   # [ci][h]
            for h in range(2):
                # stripe 0 scores the whole band; stripe 1 only its own half
                ps_a0 = pp_mm.tile([S, SC], F32, tag="mm", name=f"psa{sc}_0_{h}")
                nc.tensor.matmul(ps_a0, kfT[h][:, sub[0]], qfT[h][:, band],
                                 start=True, stop=True)
                atm0 = atm_pool.tile([S, SC], F16, tag="atm", name=f"atm{sc}_0_{h}")
                nc.vector.tensor_mul(atm0, ps_a0, m0_s)
                ps_a1 = pp_mm.tile([S, S], F32, tag="mm", name=f"psa{sc}_1_{h}")
                nc.tensor.matmul(ps_a1, kfT[h][:, sub[1]], qfT[h][:, sub[1]],
                                 start=True, stop=True)
                atm1 = atm_pool.tile([S, S], F16, tag="atm1", name=f"atm{sc}_1_{h}")
                nc.vector.tensor_mul(atm1, ps_a1, m0_s[:, 0:S])

                # ctx^T (+norm row 64) = prefix-state inter + two stripe intras
                ps_c = pp_cs.tile([D + 2, SC], F32, tag="cs", name=f"psc{sc}_{h}")
                if sc > 0:
                    nc.tensor.matmul(ps_c, state[h], qfT[h][:, band], start=True, stop=False)
                    nc.tensor.matmul(ps_c, vp[0][h], atm0, start=False, stop=False)
                    nc.tensor.matmul(ps_c[:, S:SC], vp[1][h], atm1, start=False, stop=True)
                else:
                    nc.tensor.matmul(ps_c, vp[0][h], atm0, start=True, stop=False)
                    nc.tensor.matmul(ps_c[:, S:SC], vp[1][h], atm1, start=False, stop=True)

                # state += Kf^T V' over both stripes
                ps_s = pp_cs.tile([128, D + 2], F32, tag="cs", name=f"pss{sc}_{h}")
                nc.tensor.matmul(ps_s, kfeat[0][h], vp[0][h], start=True, stop=False)
                nc.tensor.matmul(ps_s, kfeat[1][h], vp[1][h], start=False, stop=True)
                if sc == 0:
                    nc.vector.tensor_copy(state[h], ps_s)
                else:
                    nc.vector.tensor_add(state[h], state[h], ps_s)

                # norm row -> [t,1] columns (PE transpose) -> reciprocal
                nrow = nrm_pool.tile([1, SC], F16, tag="nrow", name=f"nrow{sc}_{h}")
                nc.scalar.activation(out=nrow, in_=ps_c[D:D + 1, :],
                                     func=mybir.ActivationFunctionType.Identity,
                                     bias=eps_t[0:1, 0:1], scale=1.0)
                for ci in range(2):
                    ps_n = kt_tiles[ci][:, 384 + 2 * h:386 + 2 * h]
                    nc.tensor.transpose(ps_n, nrow[:, ci * S:(ci + 1) * S], ident[0:1, 0:2])
                    nc_t = nrm_pool.tile([S, 1], F32, tag="ncol", name=f"ncol{sc}_{ci}_{h}")
                    nc.vector.reciprocal(nc_t, ps_n[:, 0:1])
                    ncol[ci][h] = nc_t

                # unnormalized ctx -> SBUF; per-stripe per-head out-projection
                ctxu = nrm_pool.tile([D, SC], F16, tag="ctxu", name=f"ctxu{sc}_{h}")
                nc.scalar.copy(ctxu, ps_c[0:D, :])
                for ci in range(2):
                    ps = pp_big.tile([128, E], F32, tag="big", name=f"pso{sc}_{ci}_{h}")
                    nc.tensor.matmul(ps, ctxu[:, ci * S:(ci + 1) * S], w2h[h],
                                     start=True, stop=True)
                    ps_o[ci][h] = ps

            # scale by 1/norm (per-partition) and combine heads
            for ci in range(2):
                o_s = osb_pool.tile([128, E], F16, tag="osb", name=f"os{sc}_{ci}")
                nc.scalar.activation(out=o_s, in_=ps_o[ci][0],
                                     func=mybir.ActivationFunctionType.Copy,
                                     scale=ncol[ci][0])
                nc.vector.scalar_tensor_tensor(
                    out=o_s, in0=ps_o[ci][1], scalar=ncol[ci][1], in1=o_s,
                    op0=mybir.AluOpType.mult, op1=mybir.AluOpType.add,
                )
                nc.sync.dma_start(out=out[sub[ci], :], in_=o_s)

    _split_multi_waits(nc)
    return nc


_PROGRAM = None


def _get_program():
    global _PROGRAM
    if _PROGRAM is None:
        _PROGRAM = build_program()
    return _PROGRAM


def _make_in_maps(x, w_qkv, b_qkv, w_out):
    pos = np.arange(T, dtype=np.float32)
    ang = (math.pi / 2) * pos / T
    cosw = np.cos(ang).astype(np.float32)
    sinw = np.sin(ang).astype(np.float32)
    csrep = np.concatenate([
        np.broadcast_to(cosw[None, :], (D, T)),
        np.broadcast_to(sinw[None, :], (D, T)),
    ], 0).astype(np.float16)
    tri = np.triu(np.ones((S, S), np.float16))
    m0 = np.concatenate([tri, np.ones((S, S), np.float16)], 1)

    in_maps = []
    for i in range(8):
        b, g = divmod(i, 4)
        h0, h1 = 2 * g, 2 * g + 1
        wq = lambda h: w_qkv[h * D:(h + 1) * D]
        wk = lambda h: w_qkv[E + h * D:E + (h + 1) * D]
        wv = lambda h: w_qkv[2 * E + h * D:2 * E + (h + 1) * D]
        bq = lambda h: b_qkv[h * D:(h + 1) * D]
        bk = lambda h: b_qkv[E + h * D:E + (h + 1) * D]
        bv = lambda h: b_qkv[2 * E + h * D:2 * E + (h + 1) * D]
        hcols = np.r_[h0 * D:(h0 + 1) * D, h1 * D:(h1 + 1) * D]
        wqkf = np.concatenate([
            wq(h0), wq(h0), wq(h1), wq(h1), wk(h0), wk(h0), wk(h1), wk(h1)
        ], 0).T
        # bias blob [128, 5]: col bi = dup'd qk bias for feature block bi,
        # col 4 = v bias (both heads stacked)
        bias_cols = [
            np.concatenate([bq(h0), bq(h0)]),
            np.concatenate([bq(h1), bq(h1)]),
            np.concatenate([bk(h0), bk(h0)]),
            np.concatenate([bk(h1), bk(h1)]),
            np.concatenate([bv(h0), bv(h1)]),
        ]
        biasblob = np.stack(bias_cols, axis=1).astype(np.float32)
        in_maps.append({
            "xt": np.ascontiguousarray(x[b].T.astype(np.float16)),
            "wqkf": np.ascontiguousarray(wqkf.astype(np.float16)),
            "wvt": np.ascontiguousarray(
                np.concatenate([wv(h0), wv(h1)], 0).T.astype(np.float16)),
            "biasblob": biasblob,
            "csrep": csrep,
            "w2": np.ascontiguousarray(w_out[:, hcols].T.astype(np.float16)),
            "identin": np.eye(128, dtype=np.float16),
            "m0in": m0,
        })
    return in_maps


def run(inputs, trace=False):
    x = np.asarray(inputs["x"], dtype=np.float32)
    w_qkv = np.asarray(inputs["w_qkv"], dtype=np.float32)
    b_qkv = np.asarray(inputs["b_qkv"], dtype=np.float32)
    w_out = np.asarray(inputs["w_out"], dtype=np.float32)
    b_out = np.asarray(inputs["b_out"], dtype=np.float32)

    nc = _get_program()
    in_maps = _make_in_maps(x, w_qkv, b_qkv, w_out)
    res = run_bass_kernel_spmd(nc, in_maps, list(range(8)), trace=trace)

    out = np.empty((B, T, E), dtype=np.float32)
    for b in range(B):
        acc = res.results[4 * b]["out"].astype(np.float32)
        for g in range(1, 4):
            acc = acc + res.results[4 * b + g]["out"].astype(np.float32)
        out[b] = acc + b_out[None, :]
    return out, res


def kernel(**inputs) -> np.ndarray:
    out, _ = run(inputs, trace=False)
    return out
